# revision 15
# baseline (speedup 1.0000x reference)
"""Brute-force KNN density estimator on 8 Trainium2 NeuronCores.

reference math:
    dist[i, j] = ||x_i - x_j||_2 over features [8192, 1024]
    kth[i] = 6th smallest of dist[i, :]  (self-distance included)
    out[i] = 1 / (kth[i] + 1e-8)

Strategy (data-parallel over query rows, 1024 rows per core):
    - Rank rows of the distance matrix by T[i,j] = 2*G[i,j] - (sq[j] - mean(sq))
      (per-row-constant sq[i] and the monotone sqrt don't change ranking).
    - TensorE: G via fp8 e4m3 DoubleRow matmuls only (157 TF/s roofline);
      the norm subtraction is OFF the PE: ScalarE downcasts each 4-bank PSUM
      group [128, 2048] to bf16 SBUF, VectorE subtracts the centered norms
      (bf16 2x mode) and runs one MAX8 per 2048-wide group.
    - Loop order g(4 column groups) -> r(8 row tiles) -> t(4 tiles): PSUM
      holds 2 groups of 4 banks; each ft column tile is reused 8x so the
      just-in-time ft DMA stream only needs ~72 GB/s.
    - Exact: per-group top-8 supersets per-group top-6; final MAX8 over the
      4 groups' candidates gives the exact 6th largest T; the kth distance
      is recovered with exact fp32 norms: kth_d2 = (sq[i] + mean(sq)) - T6.
"""

import os

import numpy as np
import ml_dtypes

N = 8192          # points
D = 1024          # feature dim
NCORES = 8
ROWS = N // NCORES   # rows (queries) per core
RT = ROWS // 128     # row tiles per core
CTILE = 512          # matmul moving free dim (one PSUM bank)
CT = N // CTILE      # column tiles
GT = 4               # column tiles per group (4 PSUM banks)
NG = CT // GT        # column groups
KC = D // 128        # 128-row contraction chunks
K_ORD = 5            # 0-based rank -> 6th smallest
EPS = 1e-8
WARMUP_MM = int(os.environ.get("KNN_WARMUP", "10"))
SUB_GPSIMD = bool(int(os.environ.get("KNN_SUB_GPSIMD", "1")))

TRACE = bool(int(os.environ.get("KNN_TRACE", "0")))
LAST_EXEC_NS = None


def _build_nc():
    import concourse.mybir as mybir
    from concourse import bacc
    from concourse.tile import TileContext

    dt = mybir.dt
    nc = bacc.Bacc(None, target_bir_lowering=False, enable_partition_id=False)

    # qt: k-major (chunk stride ROWS) — large lhsT chunk strides keep the DR
    # LDWEIGHTS fully hidden under the previous matmul's streaming
    qt_d = nc.dram_tensor("qt", [128, KC * ROWS], dt.float8e4, kind="ExternalInput")
    # ft: t-major [CT][KC][512]; first group staged per-tile, rest per-group
    ft_d = nc.dram_tensor("ft", [128, CT * KC * CTILE], dt.float8e4, kind="ExternalInput")
    sqc_d = nc.dram_tensor("sqc", [128, N], dt.bfloat16, kind="ExternalInput")
    sqi_d = nc.dram_tensor("sqi", [128, RT], dt.float32, kind="ExternalInput")
    out_d = nc.dram_tensor("out", [128, RT], dt.float32, kind="ExternalOutput")

    DR = mybir.MatmulPerfMode.DoubleRow
    GRP = GT * KC * CTILE  # ft elements per group per partition

    with TileContext(nc) as tc:
        with (
            tc.tile_pool(name="persist", bufs=1) as persist,
            tc.tile_pool(name="stage", bufs=3) as stage,
            tc.tile_pool(name="psum", bufs=2, space="PSUM") as psum,
        ):
            qt_s = persist.tile([128, KC, ROWS], dt.float8e4)
            ft0_s = [
                persist.tile([128, KC, CTILE], dt.float8e4, name=f"ft0_{t}")
                for t in range(GT)
            ]
            ftg_s = [
                persist.tile([128, GT, KC, CTILE], dt.float8e4, name=f"ftg_{g}")
                for g in range(1, NG)
            ]
            sqc_s = persist.tile([128, CT, CTILE], dt.bfloat16)
            sqi_s = persist.tile([128, RT], dt.float32)
            cand = persist.tile([128, RT, 6, 8], dt.bfloat16)
            top8 = persist.tile([128, RT, 8], dt.bfloat16)
            warm_s = persist.tile([128, CTILE], dt.bfloat16)

            # PE warm-up: keep the PE busy while the first DMAs land so the
            # HAM clock reaches 2.4 GHz before the real matmuls
            nc.vector.memset(warm_s, 0.0)
            # unwritten merge slots must lose every max8
            nc.vector.memset(cand, -1e30)
            wps = psum.tile([128, GT, CTILE], dt.float32, tag="ps")
            for i in range(WARMUP_MM):
                nc.tensor.matmul(wps[:, 0, :], lhsT=warm_s[:, 0:128], rhs=warm_s,
                                 start=(i == 0), stop=(i == WARMUP_MM - 1))

            # DMA issue order = FIFO service order: critical path first
            nc.sync.dma_start(qt_s, qt_d.rearrange("p (k i) -> p k i", k=KC))
            nc.sync.dma_start(
                ft0_s[0],
                ft_d[:, 0:KC * CTILE].rearrange("p (k j) -> p k j", k=KC),
            )
            nc.sync.dma_start(
                sqc_s[:, 0:GT, :],
                sqc_d[:, 0:GT * CTILE].rearrange("p (t j) -> p t j", j=CTILE),
            )
            for t in range(1, GT):
                nc.sync.dma_start(
                    ft0_s[t],
                    ft_d[:, t * KC * CTILE:(t + 1) * KC * CTILE]
                    .rearrange("p (k j) -> p k j", k=KC),
                )
            nc.sync.dma_start(sqi_s, sqi_d[:, :])
            nc.sync.dma_start(
                ftg_s[0],
                ft_d[:, 1 * GRP:2 * GRP]
                .rearrange("p (t k j) -> p t k j", t=GT, k=KC),
            )
            nc.sync.dma_start(
                sqc_s[:, GT:, :],
                sqc_d[:, GT * CTILE:].rearrange("p (t j) -> p t j", j=CTILE),
            )
            for g in range(2, NG):
                nc.sync.dma_start(
                    ftg_s[g - 1],
                    ft_d[:, g * GRP:(g + 1) * GRP]
                    .rearrange("p (t k j) -> p t k j", t=GT, k=KC),
                )

            # column chunks: t=0 alone (runs on partial prologue data), then
            # 3-wide, then 4-wide groups; last chunk keeps subs on DVE so the
            # tail never waits on a slow GPSIMD sub
            chunks = [
                ([0], False),
                ([1, 2, 3], True),
                ([4, 5, 6, 7], True),
                ([8, 9, 10, 11], True),
                ([12, 13, 14, 15], False),
            ]
            NCH = len(chunks)
            for ci, (tiles, alt) in enumerate(chunks):
                w = len(tiles)
                for r in range(RT):
                    ps = psum.tile([128, GT, CTILE], dt.float32, tag="ps")
                    for tt, t in enumerate(tiles):
                        rhs = ft0_s[t] if t < GT else ftg_s[t // GT - 1][:, t % GT]
                        for k in range(0, KC, 2):
                            nc.tensor.matmul(
                                ps[:, tt, :],
                                lhsT=qt_s[:, k:k + 2, r * 128:(r + 1) * 128],
                                rhs=rhs[:, k:k + 2, :],
                                start=(k == 0),
                                stop=(k == KC - 2),
                                perf_mode=DR,
                            )
                    raw = stage.tile([128, GT, CTILE], dt.bfloat16, tag="raw")
                    subo = stage.tile([128, GT, CTILE], dt.bfloat16, tag="subo")
                    t0 = tiles[0]
                    last = ci == NCH - 1
                    gp_ok = SUB_GPSIMD and r % 2 == 1 and (alt or (last and r < 6))
                    sub_eng = nc.gpsimd if gp_ok else nc.vector
                    if last and r == RT - 1:
                        # the very last group: two half-width pipelines so the
                        # tail chain overlaps the final matmuls
                        h = w // 2
                        for hi in range(2):
                            sl = slice(hi * h, (hi + 1) * h)
                            csl = slice(t0 + hi * h, t0 + (hi + 1) * h)
                            nc.scalar.activation(
                                raw[:, sl, :], ps[:, sl, :],
                                mybir.ActivationFunctionType.Copy,
                            )
                            nc.vector.tensor_sub(
                                subo[:, sl, :], raw[:, sl, :], sqc_s[:, csl, :]
                            )
                            nc.vector.max(
                                out=cand[:, r, ci + hi, :], in_=subo[:, sl, :]
                            )
                    else:
                        nc.scalar.activation(
                            raw[:, 0:w, :], ps[:, 0:w, :],
                            mybir.ActivationFunctionType.Copy,
                        )
                        sub_eng.tensor_sub(
                            subo[:, 0:w, :], raw[:, 0:w, :], sqc_s[:, t0:t0 + w, :]
                        )
                        nc.vector.max(out=cand[:, r, ci, :], in_=subo[:, 0:w, :])
                    if last:
                        nc.vector.max(out=top8[:, r, :], in_=cand[:, r])

            # epilogue: kth_d2 = (sq[i] + sbar) - T6 -> density, all rows at once
            kd = persist.tile([128, RT], dt.float32)
            nc.vector.tensor_sub(kd, sqi_s, top8[:, :, K_ORD])
            nc.vector.tensor_scalar_max(kd, kd, 0.0)
            ks = persist.tile([128, RT], dt.float32)
            nc.scalar.activation(ks, kd, mybir.ActivationFunctionType.Sqrt)
            nc.vector.tensor_scalar_add(ks, ks, EPS)
            dens = persist.tile([128, RT], dt.float32)
            nc.vector.reciprocal(dens, ks)
            nc.sync.dma_start(out_d[:, :], dens)

    nc.finalize()
    return nc


def kernel(features):
    global LAST_EXEC_NS
    from concourse.bass_utils import run_bass_kernel_spmd

    f32 = np.ascontiguousarray(np.asarray(features, dtype=np.float32))
    assert f32.shape == (N, D)

    sq = np.einsum("nd,nd->n", f32, f32, dtype=np.float32)   # exact fp32 norms
    sbar = float(sq.mean())
    ftq = f32.T.astype(ml_dtypes.float8_e4m3fn)               # [D, N] fp8
    # moving operand pre-scaled by 2 (exact in fp8) so PSUM accumulates 2*G
    ft2 = (ftq.astype(np.float32) * 2.0).astype(ml_dtypes.float8_e4m3fn)
    # [D, N] -> [128, CT*KC*512]: partition p holds, per column tile t, all
    # KC contraction chunks contiguously
    ft_host = np.ascontiguousarray(
        ft2.reshape(KC, 128, CT, CTILE).transpose(1, 2, 0, 3).reshape(128, CT * KC * CTILE)
    )
    sqc_rep = np.ascontiguousarray(
        np.broadcast_to((sq - sbar).astype(ml_dtypes.bfloat16), (128, N))
    )

    in_maps = []
    for c in range(NCORES):
        lo = c * ROWS
        # [D, ROWS] -> [128, KC, ROWS] (k-major, chunk stride = ROWS)
        qt = np.ascontiguousarray(
            ftq[:, lo:lo + ROWS].reshape(KC, 128, ROWS).transpose(1, 0, 2).reshape(128, KC * ROWS)
        )
        sqi = np.ascontiguousarray(
            (sq[lo:lo + ROWS] + sbar).reshape(RT, 128).T.astype(np.float32)
        )
        in_maps.append({
            "qt": qt,
            "ft": ft_host,
            "sqc": sqc_rep,
            "sqi": sqi,
        })

    nc = _build_nc()
    res = run_bass_kernel_spmd(nc, in_maps, core_ids=list(range(NCORES)), trace=TRACE)
    LAST_EXEC_NS = res.exec_time_ns

    # out[p, r] = density of global row  c*1024 + r*128 + p
    out = np.concatenate([r["out"].T.reshape(-1) for r in res.results])
    return out.astype(np.float32)[:, None]
